# revision 1
# baseline (speedup 1.0000x reference)
"""NeRF-NGP MLP kernel for Trainium2 (8 NeuronCores, pure data parallel).

Network (bias-free, fp32):
  sigma net: x[:, :32] -> 64 -> 64 -> (1 sigma + 15 geo)
  color net: concat(x[:, 32:48], geo) -> 64 -> 64 -> 64 -> 3
  out = [color(3), sigma(1)]   shape [N, 4]

Device strategy (per core, N_CORE = 262144 points):
  - Activations live "layout B": [channels(partitions), points(free)].
  - Every layer is matmul(psum[M,512], lhsT=W[K,M], rhs=act[K,512]).
  - The concat is algebraically fused away on the host:
      W3  = s2[:,1:] @ c0[16:,:]   (geo path, 64x64)
      W3v = c0[:16,:] placed at partition rows 32:48 (views path)
    so  h3 = relu(W3.T @ h2 + W3v.T @ x_chans)   via PSUM accumulation.
    sigma is folded into the final layer the same way:
      out4 = W6a.T @ h5 + W6b.T @ h2  with W6a=[c3|0], W6b=[0|s2[:,0]].
  - 4-way PE-array packing: 64x64 tile_position quadrants; 4 chunks of 512
    points ("u,v,w,z") advance through the layers with a rotation schedule
    that keeps every matmul's rhs in the row group its tile reads.
  - PSUM evacuation (+relu) in full 128-partition [128, 1024] ops,
    alternating ScalarE / VectorE.
  - Input is host-pre-transposed into a blocked layout so DMA bursts are
    512B/partition contiguous; output is returned blocked and un-blocked
    on the host.
"""

import numpy as np

import concourse.bacc as bacc
import concourse.mybir as mybir
import concourse.tile as tile
from concourse.bass_utils import run_bass_kernel_spmd

F32 = mybir.dt.float32
RELU = mybir.ActivationFunctionType.Relu

N_PTS = 2097152
N_CORES = 8
N_CORE = N_PTS // N_CORES      # 262144
T = 512                        # points per chunk = one PSUM bank of fp32
CHUNKS_PER_GROUP = 4
PTS_PER_GROUP = T * CHUNKS_PER_GROUP   # 2048
G = N_CORE // PTS_PER_GROUP            # 128 groups per core

# tile name -> (rhs row-group base, psum col-position base)
TILES = {"T0": (0, 0), "T2": (0, 64), "T8": (64, 0), "T10": (64, 64)}
# tile name -> which 512-wide window of the psum/h tile the result lands in
PWIN = {"T0": 0, "T2": 0, "T8": 1, "T10": 1}

# per-layer chunk->tile assignment (chunks 0..3 = u,v,w,z).  Derived so that
#  - a chunk's rhs row group always matches its tile's row group,
#  - positions at L3 equal the xt positions (views + h2 reuse),
#  - row groups at L6 equal those at L3 (h5 and h2 read together).
SCHED = [
    {0: "T0", 2: "T2", 1: "T8", 3: "T10"},   # L1
    {0: "T0", 1: "T2", 2: "T8", 3: "T10"},   # L2
    {0: "T0", 2: "T2", 1: "T8", 3: "T10"},   # L3 (2 matmuls per chunk)
    {1: "T0", 0: "T2", 2: "T8", 3: "T10"},   # L4
    {2: "T0", 1: "T2", 0: "T8", 3: "T10"},   # L5
    {0: "T0", 2: "T2", 1: "T8", 3: "T10"},   # L6 (2 matmuls per chunk)
]

# weight free-dim offsets inside the [128, 512] weight tile
WCOL = {"W1": 0, "W3v": 64, "W2": 128, "W3": 192, "W4": 256, "W5": 320,
        "W6a": 384, "W6b": 448}

_PROG = {}


def _build_program(g_count):
    nc = bacc.Bacc()
    xp = nc.dram_tensor("xp", [g_count, 2, 48, 2, T], F32, kind="ExternalInput")
    wt = nc.dram_tensor("wt", [128, 512], F32, kind="ExternalInput")
    od = nc.dram_tensor("od", [g_count, 2, 4, 2, T], F32, kind="ExternalOutput")

    with tile.TileContext(nc) as tc:
        with (
            tc.tile_pool(name="wp", bufs=1) as wp,
            tc.tile_pool(name="xtp", bufs=3) as xtp,
            tc.tile_pool(name="h1p", bufs=3) as h1p,
            tc.tile_pool(name="h2p", bufs=3) as h2p,
            tc.tile_pool(name="h3p", bufs=3) as h3p,
            tc.tile_pool(name="h4p", bufs=3) as h4p,
            tc.tile_pool(name="h5p", bufs=3) as h5p,
            tc.tile_pool(name="osp", bufs=3) as osp,
            tc.tile_pool(name="pp", bufs=4, space="PSUM") as pp,
        ):
            w = wp.tile([128, 512], F32)
            nc.sync.dma_start(out=w, in_=wt[:, :])

            def wsl(name, rg, k0, k):
                c = WCOL[name]
                return w[rg + k0: rg + k, c: c + 64]

            for g in range(g_count):
                xt = xtp.tile([128, 2, T], F32)
                nc.sync.dma_start(out=xt[0:48], in_=xp[g, 0])
                nc.sync.dma_start(out=xt[64:112], in_=xp[g, 1])

                # chunk -> (row-group, window) position of its current act
                pos = {0: (0, 0), 1: (64, 0), 2: (0, 1), 3: (64, 1)}
                xt_pos = dict(pos)
                h2_pos = None

                hs = []   # h tiles per layer
                for L in range(6):
                    ps = pp.tile([128, 2, T], F32)
                    prev = hs[L - 1] if L > 0 else None
                    for c, tname in SCHED[L].items():
                        rg, colpos = TILES[tname]
                        pwin = PWIN[tname]
                        crg, cwin = pos[c]
                        assert crg == rg, (g, L, c, tname, pos)
                        out_ap = ps[colpos: colpos + 64, pwin]
                        if L == 0:
                            nc.tensor.matmul(
                                out=out_ap,
                                lhsT=wsl("W1", rg, 0, 48),
                                rhs=xt[crg: crg + 48, cwin],
                                start=True, stop=True,
                                tile_position=(rg, colpos),
                            )
                        elif L == 2:
                            nc.tensor.matmul(
                                out=out_ap,
                                lhsT=wsl("W3", rg, 0, 64),
                                rhs=hs[1][crg: crg + 64, cwin],
                                start=True, stop=False,
                                tile_position=(rg, colpos),
                            )
                            xrg, xwin = xt_pos[c]
                            assert (xrg, xwin) == (crg, cwin)
                            nc.tensor.matmul(
                                out=out_ap,
                                lhsT=wsl("W3v", rg, 0, 48),
                                rhs=xt[xrg: xrg + 48, xwin],
                                start=False, stop=True,
                                tile_position=(rg, colpos),
                            )
                        elif L == 5:
                            nc.tensor.matmul(
                                out=out_ap,
                                lhsT=wsl("W6a", rg, 0, 64),
                                rhs=hs[4][crg: crg + 64, cwin],
                                start=True, stop=False,
                                tile_position=(rg, colpos),
                            )
                            h2rg, h2win = h2_pos[c]
                            assert h2rg == crg
                            nc.tensor.matmul(
                                out=out_ap,
                                lhsT=wsl("W6b", rg, 0, 64),
                                rhs=hs[1][h2rg: h2rg + 64, h2win],
                                start=False, stop=True,
                                tile_position=(rg, colpos),
                            )
                        else:
                            wname = {1: "W2", 3: "W4", 4: "W5"}[L]
                            nc.tensor.matmul(
                                out=out_ap,
                                lhsT=wsl(wname, rg, 0, 64),
                                rhs=prev[crg: crg + 64, cwin],
                                start=True, stop=True,
                                tile_position=(rg, colpos),
                            )
                        pos[c] = (colpos, pwin)

                    if L < 5:
                        pool = [h1p, h2p, h3p, h4p, h5p][L]
                        h = pool.tile([128, 2, T], F32)
                        if L in (0, 2, 4):
                            nc.scalar.activation(h[:, :, :], ps[:, :, :], RELU)
                        else:
                            nc.vector.tensor_scalar_max(h[:, :, :], ps[:, :, :], 0.0)
                        hs.append(h)
                        if L == 1:
                            h2_pos = dict(pos)
                    else:
                        osb = osp.tile([128, 2, T], F32)
                        nc.vector.tensor_copy(osb[:, :, :], ps[:, :, :])
                        nc.sync.dma_start(out=od[g, 0], in_=osb[0:4])
                        nc.sync.dma_start(out=od[g, 1], in_=osb[64:68])

    nc.finalize()
    return nc


def _get_program():
    if "nc" not in _PROG:
        _PROG["nc"] = _build_program(G)
    return _PROG["nc"]


def _build_weights(s0, s1, s2, c0, c1, c2, c3):
    w = np.zeros((64, 512), np.float32)
    w[0:32, WCOL["W1"]: WCOL["W1"] + 64] = s0
    w[32:48, WCOL["W3v"]: WCOL["W3v"] + 64] = c0[:16]
    w[0:64, WCOL["W2"]: WCOL["W2"] + 64] = s1
    w[0:64, WCOL["W3"]: WCOL["W3"] + 64] = (
        s2[:, 1:].astype(np.float64) @ c0[16:].astype(np.float64)
    ).astype(np.float32)
    w[0:64, WCOL["W4"]: WCOL["W4"] + 64] = c1
    w[0:64, WCOL["W5"]: WCOL["W5"] + 64] = c2
    w[0:64, WCOL["W6a"]: WCOL["W6a"] + 3] = c3
    w[0:64, WCOL["W6b"] + 3] = s2[:, 0]
    return np.concatenate([w, w], axis=0)


def kernel(x, s0, s1, s2, c0, c1, c2, c3):
    x = np.asarray(x, dtype=np.float32)
    assert x.shape == (N_PTS, 48), x.shape
    args = [np.asarray(a, dtype=np.float32) for a in (s0, s1, s2, c0, c1, c2, c3)]
    w_host = _build_weights(*args)

    in_maps = []
    for i in range(N_CORES):
        xc = x[i * N_CORE: (i + 1) * N_CORE]
        xprep = np.ascontiguousarray(
            xc.reshape(G, 2, 2, T, 48).transpose(0, 2, 4, 1, 3)
        )
        in_maps.append({"xp": xprep, "wt": w_host})

    nc = _get_program()
    res = run_bass_kernel_spmd(nc, in_maps, core_ids=list(range(N_CORES)))

    outs = []
    for i in range(N_CORES):
        od = res.results[i]["od"]
        outs.append(od.transpose(0, 1, 3, 4, 2).reshape(N_CORE, 4))
    return np.concatenate(outs, axis=0)


# revision 11
# speedup vs baseline: 485.8857x; 485.8857x over previous
"""NeRF-NGP MLP kernel for Trainium2 (8 NeuronCores, pure data parallel).

Network (bias-free, fp32 reference):
  sigma net: x[:, :32] -> 64 -> 64 -> (1 sigma + 15 geo)
  color net: concat(x[:, 32:48], geo) -> 64 -> 64 -> 64 -> 3
  out = [color(3), sigma(1)]   shape [N, 4]

Device strategy (per core, N_CORE = 262144 points):
  - Activations live "layout B": [channels(partitions), points(free)].
  - Every layer is matmul(psum[M,512], lhsT=W[K,M], rhs=act[K,512]).
  - The concat is algebraically fused away on the host:
      W3  = s2[:,1:] @ c0[16:,:]   (geo path, 64x64)
      W3v = c0[:16,:] placed at partition rows 32:48 (views path)
    so  h3 = relu(W3.T @ h2 + W3v.T @ x_chans)   via PSUM accumulation.
    sigma is folded into the final layer the same way:
      out4 = W6a.T @ h5 + W6b.T @ h2  with W6a=[c3|0], W6b=[0|s2[:,0]].
  - 4-way PE-array packing: 64x64 tile_position quadrants; 4 chunks of 512
    points ("u,v,w,z") advance through the layers with a rotation schedule
    that keeps every matmul's rhs in the row group its tile reads.
  - Matmul operands in fp16 (1 cyc/col on the PE vs 4 for fp32; 11-bit
    mantissa adds ~6e-4 absmax-relative error end to end). PSUM stays fp32.
  - PSUM evacuation (+relu) in full 128-partition [128, 1024] ops,
    alternating ScalarE / VectorE per layer.
  - ILV groups are software-pipelined (emission round-robin) so each
    engine's in-order stream has independent work during evac waits.
  - Input is host-pre-transposed into a blocked layout so DMA bursts are
    contiguous per partition; output is returned blocked and un-blocked
    on the host.
"""

import numpy as np

import concourse.bacc as bacc
import concourse.mybir as mybir
import concourse.tile as tile
from concourse.bass_utils import run_bass_kernel_spmd

F32 = mybir.dt.float32
RELU = mybir.ActivationFunctionType.Relu

N_PTS = 2097152
N_CORES = 8
N_CORE = N_PTS // N_CORES      # 262144
T = 512                        # points per chunk = one PSUM bank of fp32
CHUNKS_PER_GROUP = 4
PTS_PER_GROUP = T * CHUNKS_PER_GROUP   # 2048
G = N_CORE // PTS_PER_GROUP            # 128 groups per core

# matmul operand dtype: float16 (1 cyc/col) or float32 (exact, 4 cyc/col)
MM_DT = mybir.dt.float16
ILV = 4        # groups software-pipelined together

# tile name -> (rhs row-group base, psum col-position base)
TILES = {"T0": (0, 0), "T2": (0, 64), "T8": (64, 0), "T10": (64, 64)}
# tile name -> which 512-wide window of the psum/h tile the result lands in
PWIN = {"T0": 0, "T2": 0, "T8": 1, "T10": 1}

# per-layer chunk->tile assignment (chunks 0..3 = u,v,w,z).  Derived so that
#  - a chunk's rhs row group always matches its tile's row group,
#  - positions at L3 equal the xt positions (views + h2 reuse),
#  - row groups at L6 equal those at L3 (h5 and h2 read together).
SCHED = [
    {0: "T0", 2: "T2", 1: "T8", 3: "T10"},   # L1
    {0: "T0", 1: "T2", 2: "T8", 3: "T10"},   # L2
    {0: "T0", 2: "T2", 1: "T8", 3: "T10"},   # L3 (2 matmuls per chunk)
    {1: "T0", 0: "T2", 2: "T8", 3: "T10"},   # L4
    {2: "T0", 1: "T2", 0: "T8", 3: "T10"},   # L5
    {0: "T0", 2: "T2", 1: "T8", 3: "T10"},   # L6 (2 matmuls per chunk)
]

# weight free-dim offsets inside the [128, 512] weight tile
WCOL = {"W1": 0, "W3v": 64, "W2": 128, "W3": 192, "W4": 256, "W5": 320,
        "W6a": 384, "W6b": 448}

_PROG = {}


def _np_mm_dt():
    return np.float16 if MM_DT == mybir.dt.float16 else np.float32


def _build_program(g_count, passes=1, hbufs=None, xbufs=None, obufs=3,
                   pbufs=4, ilv=None):
    if ilv is None:
        ilv = ILV
    if hbufs is None:
        hbufs = ilv + 1
    if xbufs is None:
        xbufs = ilv + 1
    mdt = MM_DT
    nc = bacc.Bacc()
    xp = nc.dram_tensor("xp", [g_count, 2, 48, 2, T], mdt, kind="ExternalInput")
    wt = nc.dram_tensor("wt", [128, 512], mdt, kind="ExternalInput")
    od = nc.dram_tensor("od", [g_count, 2, 4, 2, T], F32, kind="ExternalOutput")

    with tile.TileContext(nc) as tc:
        with (
            tc.tile_pool(name="wp", bufs=1) as wp,
            tc.tile_pool(name="xtp", bufs=xbufs) as xtp,
            tc.tile_pool(name="h1p", bufs=hbufs) as h1p,
            tc.tile_pool(name="h2p", bufs=hbufs) as h2p,
            tc.tile_pool(name="h3p", bufs=hbufs) as h3p,
            tc.tile_pool(name="h4p", bufs=hbufs) as h4p,
            tc.tile_pool(name="h5p", bufs=hbufs) as h5p,
            tc.tile_pool(name="osp", bufs=obufs) as osp,
            tc.tile_pool(name="pp", bufs=pbufs, space="PSUM") as pp,
        ):
            hpools = [h1p, h2p, h3p, h4p, h5p]
            w = wp.tile([128, 512], mdt)
            nc.sync.dma_start(out=w, in_=wt[:, :])

            def wsl(name, rg, k):
                c = WCOL[name]
                return w[rg: rg + k, c: c + 64]

            def emit_step(st, L, g):
                ps = pp.tile([128, 2, T], F32)
                xt, pos, hs = st["xt"], st["pos"], st["hs"]
                prev = hs[L - 1] if L > 0 else None
                for c, tname in SCHED[L].items():
                    rg, colpos = TILES[tname]
                    pwin = PWIN[tname]
                    crg, cwin = pos[c]
                    assert crg == rg, (g, L, c, tname, pos)
                    out_ap = ps[colpos: colpos + 64, pwin]
                    tp = (rg, colpos)
                    if L == 0:
                        nc.tensor.matmul(
                            out=out_ap, lhsT=wsl("W1", rg, 48),
                            rhs=xt[crg: crg + 48, cwin],
                            start=True, stop=True, tile_position=tp)
                    elif L == 2:
                        nc.tensor.matmul(
                            out=out_ap, lhsT=wsl("W3", rg, 64),
                            rhs=hs[1][crg: crg + 64, cwin],
                            start=True, stop=False, tile_position=tp)
                        xrg, xwin = st["xt_pos"][c]
                        assert (xrg, xwin) == (crg, cwin)
                        nc.tensor.matmul(
                            out=out_ap, lhsT=wsl("W3v", rg, 48),
                            rhs=xt[xrg: xrg + 48, xwin],
                            start=False, stop=True, tile_position=tp)
                    elif L == 5:
                        nc.tensor.matmul(
                            out=out_ap, lhsT=wsl("W6a", rg, 64),
                            rhs=hs[4][crg: crg + 64, cwin],
                            start=True, stop=False, tile_position=tp)
                        h2rg, h2win = st["h2_pos"][c]
                        assert h2rg == crg
                        nc.tensor.matmul(
                            out=out_ap, lhsT=wsl("W6b", rg, 64),
                            rhs=hs[1][h2rg: h2rg + 64, h2win],
                            start=False, stop=True, tile_position=tp)
                    else:
                        wname = {1: "W2", 3: "W4", 4: "W5"}[L]
                        nc.tensor.matmul(
                            out=out_ap, lhsT=wsl(wname, rg, 64),
                            rhs=prev[crg: crg + 64, cwin],
                            start=True, stop=True, tile_position=tp)
                    pos[c] = (colpos, pwin)

                if L < 5:
                    h = hpools[L].tile([128, 2, T], mdt)
                    on_act = L in (0, 2, 4) or (L == 3 and g % 3 == 0)
                    if on_act:
                        nc.scalar.activation(h[:, :, :], ps[:, :, :], RELU)
                    else:
                        nc.vector.tensor_scalar_max(h[:, :, :], ps[:, :, :], 0.0)
                    hs.append(h)
                    if L == 1:
                        st["h2_pos"] = dict(pos)
                else:
                    osb = osp.tile([128, 2, T], F32)
                    nc.vector.tensor_copy(osb[:, :, :], ps[:, :, :])
                    nc.sync.dma_start(out=od[g, 0], in_=osb[0:4])
                    nc.sync.dma_start(out=od[g, 1], in_=osb[64:68])

            glist = [g for _ in range(passes) for g in range(g_count)]
            for gbase in range(0, len(glist), ilv):
                block = glist[gbase: gbase + ilv]
                st = {}
                for g in block:
                    xt = xtp.tile([128, 2, T], mdt)
                    nc.sync.dma_start(out=xt[0:48], in_=xp[g, 0])
                    nc.sync.dma_start(out=xt[64:112], in_=xp[g, 1])
                    st[g] = {
                        "xt": xt,
                        "pos": {0: (0, 0), 1: (64, 0), 2: (0, 1), 3: (64, 1)},
                        "hs": [],
                    }
                    st[g]["xt_pos"] = dict(st[g]["pos"])
                for L in range(6):
                    for g in block:
                        emit_step(st[g], L, g)

    nc.finalize()
    return nc


def _get_program():
    if "nc" not in _PROG:
        _PROG["nc"] = _build_program(G)
    return _PROG["nc"]


def _build_weights(s0, s1, s2, c0, c1, c2, c3):
    w = np.zeros((64, 512), np.float32)
    w[0:32, WCOL["W1"]: WCOL["W1"] + 64] = s0
    w[32:48, WCOL["W3v"]: WCOL["W3v"] + 64] = c0[:16]
    w[0:64, WCOL["W2"]: WCOL["W2"] + 64] = s1
    w[0:64, WCOL["W3"]: WCOL["W3"] + 64] = (
        s2[:, 1:].astype(np.float64) @ c0[16:].astype(np.float64)
    ).astype(np.float32)
    w[0:64, WCOL["W4"]: WCOL["W4"] + 64] = c1
    w[0:64, WCOL["W5"]: WCOL["W5"] + 64] = c2
    w[0:64, WCOL["W6a"]: WCOL["W6a"] + 3] = c3
    w[0:64, WCOL["W6b"] + 3] = s2[:, 0]
    return np.concatenate([w, w], axis=0)


def kernel(x, s0, s1, s2, c0, c1, c2, c3):
    x = np.asarray(x, dtype=np.float32)
    assert x.shape == (N_PTS, 48), x.shape
    args = [np.asarray(a, dtype=np.float32) for a in (s0, s1, s2, c0, c1, c2, c3)]
    w_host = _build_weights(*args).astype(_np_mm_dt())

    in_maps = []
    for i in range(N_CORES):
        xc = x[i * N_CORE: (i + 1) * N_CORE]
        xprep = np.ascontiguousarray(
            xc.reshape(G, 2, 2, T, 48).transpose(0, 2, 4, 1, 3)
        ).astype(_np_mm_dt())
        in_maps.append({"xp": xprep, "wt": w_host})

    nc = _get_program()
    res = run_bass_kernel_spmd(nc, in_maps, core_ids=list(range(N_CORES)))

    outs = []
    for i in range(N_CORES):
        od = res.results[i]["od"]
        outs.append(od.transpose(0, 1, 3, 4, 2).reshape(N_CORE, 4))
    return np.concatenate(outs, axis=0)
